# revision 16
# baseline (speedup 1.0000x reference)
"""Trainium2 Bass kernel for ChamferkNNDist.

Problem (B=8, N=4096, 3-D points):
  chamfer = mean_b mean_j min_i ||adv[b,j] - ori[b,i]||^2
  knn: per adv point, mean of its 5 nearest-neighbour sq-distances within
       adv[b] (excluding self), then a mean+1.05*std threshold mask.
  out = 5*chamfer + 3*knn_loss  (fp32 scalar)

Strategy: data-parallel over batch, one batch element per NeuronCore (8 cores).
Per core, both N x N squared-distance matrices are computed as a single K=5
augmented fp32 matmul producing -D directly:
    -||x-y||^2 = [2x, -|x|^2, -1] . [y, 1, |y|^2]
PE streams 1 column/cycle regardless of K, so this costs the same as any
matmul with 4096x4096 output. The kNN top-6 uses the VectorEngine MAX8
instruction (8 largest per partition, descending) reading PSUM directly;
chamfer row-minima use reduce_max on -D. Tiny per-row results (top8 [N,8],
rowmax [N]) are DMA'd out and the final mean/std/threshold reduction is done
on host in fp64.
"""

import sys

if "/opt/trn_rl_repo" not in sys.path:
    sys.path.insert(0, "/opt/trn_rl_repo")

from contextlib import ExitStack

import numpy as np

import concourse.bacc as bacc
import concourse.tile as tile
from concourse import mybir
from concourse.bass_utils import run_bass_kernel_spmd

F32 = mybir.dt.float32
F32R = mybir.dt.float32r
BF16 = mybir.dt.bfloat16
F16 = mybir.dt.float16
CH16 = False  # route the chamfer fold through fp16 SBUF (DVE 2x mode)
OUT_BUFS = 2  # SBUF working-tile double/triple buffering
B = 8
N = 4096
NCORES = 8
QW = 1024  # PSUM quarter width (2 banks); 4 rotating slots cover all 8 banks
VARIANT = "bf16x2"  # "f32r" | "bf16x2" | "f32"

CHAMFER_W = 5.0
KNN_W = 3.0
KNN_K = 5
KNN_ALPHA = 1.05


def build_program(n=N, reps=1, variant=None):
    """Bass program for one core: one batch element of size n.

    Inputs (host-prepared):
      f32/f32r variant: ua/va/vo [5, n] fp32
        ua: per adv point j the row [2a, -|a|^2, -1]  (matmul weights)
        va: per adv point j the row [a, 1, |a|^2]     (kNN moving operand)
        vo: per ori point i the row [o, 1, |o|^2]     (chamfer moving op)
      bf16x2 variant: ua/va/vo [15, n] bf16 hi/lo splits:
        ua = [u_hi; u_hi; u_lo], va = [v_hi; v_lo; v_hi], vo likewise, so a
        single K=15 bf16 matmul computes u_hi.v_hi + u_hi.v_lo + u_lo.v_hi.
    Outputs:
      top8 [n, 8]: 8 largest values of -D_adv,adv per row (descending)
      cmax [n]: max_i of -||a_j - o_i||^2 per row
    reps > 1 wraps the whole body in a hardware loop (for timing only).
    """
    variant = variant or VARIANT
    nt = n // 128
    kdim = 15 if variant == "bf16x2" else 5
    in_dt = BF16 if variant == "bf16x2" else F32
    nc = bacc.Bacc("TRN2", target_bir_lowering=False, debug=False)
    ua = nc.dram_tensor("ua", [kdim, n], in_dt, kind="ExternalInput").ap()
    va = nc.dram_tensor("va", [kdim, n], in_dt, kind="ExternalInput").ap()
    vo = nc.dram_tensor("vo", [kdim, n], in_dt, kind="ExternalInput").ap()
    top8 = nc.dram_tensor("top8", [n, 8], F32, kind="ExternalOutput").ap()
    cmax = nc.dram_tensor("cmax", [n], F32, kind="ExternalOutput").ap()

    with tile.TileContext(nc) as tc:
        with ExitStack() as ctx:
            const_pool = ctx.enter_context(tc.tile_pool(name="const", bufs=1))
            psum_pool = ctx.enter_context(
                tc.tile_pool(name="ps", bufs=2, space="PSUM")
            )
            out_pool = ctx.enter_context(tc.tile_pool(name="out", bufs=OUT_BUFS))

            ua_in = const_pool.tile([kdim, n], in_dt)
            nc.sync.dma_start(ua_in[:], ua)
            va_in = const_pool.tile([kdim, n], in_dt)
            nc.sync.dma_start(va_in[:], va)
            vo_in = const_pool.tile([kdim, n], in_dt)
            nc.sync.dma_start(vo_in[:], vo)

            if variant == "f32r":
                # fp32r operands must be produced by a rounding compute op
                ua_sb = const_pool.tile([kdim, n], F32R)
                nc.vector.tensor_copy(ua_sb[:], ua_in[:])
                va_sb = const_pool.tile([kdim, n], F32R)
                nc.vector.tensor_copy(va_sb[:], va_in[:])
                vo_sb = const_pool.tile([kdim, n], F32R)
                nc.vector.tensor_copy(vo_sb[:], vo_in[:])
            else:
                ua_sb, va_sb, vo_sb = ua_in, va_in, vo_in

            HW_ = n // 2  # half width in columns

            def body(_i=None):
                ch_dt = F16 if CH16 else F32
                for t in range(nt):
                    lhsT = ua_sb[:, t * 128 : (t + 1) * 128]
                    t8cat = out_pool.tile([128, 16], F32, tag="t8cat")
                    c0sb = out_pool.tile([128, HW_], ch_dt, tag="c0sb")
                    if CH16:
                        c1sb = out_pool.tile([128, HW_], ch_dt, tag="c1sb")
                    m = out_pool.tile([128, HW_], ch_dt, tag="m")
                    for h in range(2):
                        # kNN half: -D_adv,adv -> top-8 of the half via MAX8
                        pk = psum_pool.tile([128, HW_], F32, tag="ps")
                        for j in range(HW_ // 512):
                            c0 = h * HW_ + j * 512
                            nc.tensor.matmul(
                                pk[:, j * 512 : (j + 1) * 512],
                                lhsT,
                                va_sb[:, c0 : c0 + 512],
                                start=True,
                                stop=True,
                            )
                        nc.vector.max(t8cat[:, h * 8 : (h + 1) * 8], pk[:])
                        # chamfer half: -D_adv,ori
                        pc = psum_pool.tile([128, HW_], F32, tag="ps")
                        for j in range(HW_ // 512):
                            c0 = h * HW_ + j * 512
                            nc.tensor.matmul(
                                pc[:, j * 512 : (j + 1) * 512],
                                lhsT,
                                vo_sb[:, c0 : c0 + 512],
                                start=True,
                                stop=True,
                            )
                        if h == 0:
                            # stash half 0 in SBUF (ScalarE) so the DVE can
                            # consume both halves in one tensor_tensor pass
                            nc.scalar.copy(c0sb[:], pc[:])
                        elif CH16:
                            # downcast half 1 too; 16-bit TT runs at 2x
                            nc.scalar.copy(c1sb[:], pc[:])
                            nc.vector.tensor_tensor(
                                m[:], c1sb[:], c0sb[:], op=mybir.AluOpType.max
                            )
                        else:
                            nc.vector.tensor_tensor(
                                m[:], pc[:], c0sb[:], op=mybir.AluOpType.max
                            )
                    # merge: top-8 of the two half-top-8s; row max of m
                    t8 = out_pool.tile([128, 8], F32, tag="t8")
                    nc.vector.max(t8[:], t8cat[:])
                    cm = out_pool.tile([128, 1], F32, tag="cm")
                    if CH16:
                        scr = out_pool.tile([128, HW_ // 2], ch_dt, tag="scr")
                        nc.vector.tensor_tensor(
                            scr[:],
                            m[:, : HW_ // 2],
                            m[:, HW_ // 2 :],
                            op=mybir.AluOpType.max,
                        )
                        nc.vector.tensor_reduce(
                            cm[:],
                            scr[:],
                            axis=mybir.AxisListType.X,
                            op=mybir.AluOpType.max,
                        )
                    else:
                        nc.vector.tensor_reduce(
                            cm[:],
                            m[:],
                            axis=mybir.AxisListType.X,
                            op=mybir.AluOpType.max,
                        )
                    nc.sync.dma_start(top8[t * 128 : (t + 1) * 128, :], t8[:])
                    nc.sync.dma_start(cmax[t * 128 : (t + 1) * 128], cm[:])

            if reps == 1:
                body()
            else:
                with tc.For_i(0, reps, 1):
                    body()
    nc.compile()
    return nc


def make_inputs(adv_pc, ori_pc, variant=None):
    """Per-core input dicts: augmented matmul operand matrices."""
    variant = variant or VARIANT
    adv = np.asarray(adv_pc, dtype=np.float32)
    ori = np.asarray(ori_pc, dtype=np.float32)
    in_maps = []
    for b in range(B):
        a, o = adv[b], ori[b]
        na = (a * a).sum(1, dtype=np.float32)[None, :]
        no = (o * o).sum(1, dtype=np.float32)[None, :]
        one = np.ones((1, a.shape[0]), np.float32)
        ua = np.concatenate([2.0 * a.T, -na, -one], 0).astype(np.float32)
        va = np.concatenate([a.T, one, na], 0).astype(np.float32)
        vo = np.concatenate([o.T, one, no], 0).astype(np.float32)
        if variant == "bf16x2":
            import ml_dtypes

            bf = ml_dtypes.bfloat16

            def split15(m, kind):
                hi = m.astype(bf)
                lo = (m - hi.astype(np.float32)).astype(bf)
                if kind == "u":
                    return np.concatenate([hi, hi, lo], 0)
                return np.concatenate([hi, lo, hi], 0)

            in_maps.append(
                {
                    "ua": split15(ua, "u"),
                    "va": split15(va, "v"),
                    "vo": split15(vo, "v"),
                }
            )
        else:
            in_maps.append({"ua": ua, "va": va, "vo": vo})
    return in_maps


def finalize(results):
    """Host-side (fp64) final reduction from per-core top8/cmax outputs."""
    loss1 = np.empty(B, np.float64)
    knn = np.empty(B, np.float64)
    for b in range(B):
        top8 = results[b]["top8"].astype(np.float64)  # [N, 8] of -D, descending
        cmax = results[b]["cmax"].astype(np.float64)  # [N] of max(-D)
        loss1[b] = (-cmax).mean()
        # rank 0 is the self-distance; ranks 1..5 are the 5-NN sq-distances
        value = -top8[:, 1 : KNN_K + 1].mean(axis=1)
        mean = value.mean()
        std = value.std(ddof=1)
        thresh = mean + KNN_ALPHA * std
        knn[b] = (value * (value > thresh)).mean()
    total = CHAMFER_W * loss1.mean() + KNN_W * knn.mean()
    return np.float32(total)


_program_cache = {}


def kernel(adv_pc, ori_pc):
    key = "main"
    if key not in _program_cache:
        _program_cache[key] = build_program()
    nc = _program_cache[key]
    in_maps = make_inputs(adv_pc, ori_pc)
    res = run_bass_kernel_spmd(nc, in_maps, core_ids=list(range(NCORES)))
    return finalize(res.results)
